# revision 5
# baseline (speedup 1.0000x reference)
"""Trainium2 Bass kernel for BertAttention (cross-attention).

B=2, S=2048, HIDDEN=768, 12 heads x head_dim 64, float32.

Sharding: (batch, head) pairs = 2*12 = 24 units over 8 cores -> each core
handles batch (core//4) and 3 heads (3*(core%4) .. +2). Attention is
head-local so there is no cross-core traffic.

Per-core pipeline (everything f32; matmuls run in float32r mode):
  - host pre-transposes activations/weights so the device contracts the
    hidden dim directly (hsT/ctxT: [768, 2048], wT: [768, 192])
  - Q^T, K^T computed in [d, seq] layout, 2 heads packed per 128
    partitions; V^T computed then PE-transposed to V [k, d] with a ones
    column appended (-> PV matmul also produces the softmax denominator)
  - attention without max-subtraction: S^T[k, q] per 128-k-chunk,
    ACT exp(scale*x + mask[k]) with the mask folded in as per-partition
    bias, PV accumulates ctx^T (+ denominator) in PSUM over k-chunks
  - epilogue: PE-transpose ctx^T -> ctx[q, d], DVE reciprocal +
    per-partition scale by 1/denominator, DMA out.
"""

import numpy as np

import concourse.bass as bass
import concourse.mybir as mybir
import concourse.tile as tile
from concourse import bacc
from concourse.bass_utils import run_bass_kernel_spmd
from concourse.masks import make_identity

S = 2048
HIDDEN = 768
D = 64
B = 2
NCORES = 8
HPC = 3                  # heads per core
RPC = HPC * D            # 192 weight rows / output cols per core
HC = HIDDEN // 128       # 6 hidden chunks
KC = S // 128            # 16 key chunks
NQ = S // 512            # 4 query chunks of 512
SCALE = float(1.0 / np.sqrt(D + 1e-5))
F32 = mybir.dt.float32
F32R = mybir.dt.float32r
EXP = mybir.ActivationFunctionType.Exp

_cached_nc = None


def _emit(tc, nc, d):
    """Emit the whole per-core program. d maps dram tensor names -> APs."""
    cst = tc.alloc_tile_pool(name="cst", bufs=1)
    qkv = tc.alloc_tile_pool(name="qkv", bufs=1)
    outp = tc.alloc_tile_pool(name="outp", bufs=1)

    ident = cst.tile([128, 128], F32, tag="ident")
    make_identity(nc, ident[:])
    mask_sb = cst.tile([128, KC], F32, tag="mask")
    nc.sync.dma_start(mask_sb[:], d["mask"][:])
    bias_ab = {}
    bias_c = {}
    for w in ("bq", "bk", "bv"):
        bias_ab[w] = cst.tile([128, 1], F32, tag=f"{w}ab", name=f"{w}ab")
        nc.sync.dma_start(bias_ab[w][:], d[w][0:128, :])
        bias_c[w] = cst.tile([64, 1], F32, tag=f"{w}c", name=f"{w}c")
        nc.sync.dma_start(bias_c[w][:], d[w][128:RPC, :])

    # persistent activation tensors
    qt_ab = qkv.tile([128, S], F32R, tag="qt_ab")   # Q^T heads 0,1 (d on partitions)
    qt_c = qkv.tile([64, S], F32R, tag="qt_c")      # Q^T head 2
    kt_ab = qkv.tile([128, S], F32R, tag="kt_ab")
    kt_c = qkv.tile([64, S], F32R, tag="kt_c")
    # V in [k, d] layout: per k-chunk 3 blocks of 65 (64 dims + ones col)
    v_sb = qkv.tile([128, KC, 3 * 65], F32R, tag="v_sb")
    ones_dst = v_sb[:, :, 0:195].rearrange("p k (h x) -> p k h x", h=3)[:, :, :, 64:65]
    nc.sync.dma_start(ones_dst, d["ones"][:, :].rearrange("p (k h x) -> p k h x", k=KC, h=3))

    out_sb = outp.tile([128, KC, RPC], F32, tag="out_sb")

    # ---- phase A: load + QKV projections -------------------------------
    with tc.tile_pool(name="acts", bufs=1) as acts, \
         tc.tile_pool(name="wts", bufs=1) as wts, \
         tc.tile_pool(name="tmpA", bufs=1) as tmpA, \
         tc.tile_pool(name="psA", space="PSUM", bufs=1) as psA:

        hsT = acts.tile([128, HC, S], F32R, tag="hsT")
        cxT = acts.tile([128, HC, S], F32R, tag="cxT")
        for c in range(HC):
            nc.sync.dma_start(hsT[:, c, :], d["hsT"][c * 128:(c + 1) * 128, :])
            nc.sync.dma_start(cxT[:, c, :], d["ctxT"][c * 128:(c + 1) * 128, :])
        wT = {}
        for w in ("wq", "wk", "wv"):
            wT[w] = wts.tile([128, HC, RPC], F32R, tag=f"{w}T", name=f"{w}T")
            for c in range(HC):
                nc.sync.dma_start(wT[w][:, c, :],
                                  d[w + "T"][c * 128:(c + 1) * 128, :])

        # Q^T and K^T projections: out[d, q] = sum_hid W^T[hid, d]^T hsT[hid, q]
        for w, rhs, dst_ab, dst_c in (("wq", hsT, qt_ab, qt_c),
                                      ("wk", cxT, kt_ab, kt_c)):
            for qc in range(NQ):
                ps = psA.tile([128, 512], F32, tag="proj", bufs=2)
                for c in range(HC):
                    nc.tensor.matmul(ps[:, :],
                                     lhsT=wT[w][:, c, 0:128],
                                     rhs=rhs[:, c, qc * 512:(qc + 1) * 512],
                                     start=(c == 0), stop=(c == HC - 1))
                nc.vector.tensor_scalar_add(
                    dst_ab[:, qc * 512:(qc + 1) * 512], ps[:, :], bias_ab[w.replace("w", "b")][:])
                ps2 = psA.tile([64, 512], F32, tag="projc", bufs=2)
                for c in range(HC):
                    nc.tensor.matmul(ps2[:, :],
                                     lhsT=wT[w][:, c, 128:RPC],
                                     rhs=rhs[:, c, qc * 512:(qc + 1) * 512],
                                     start=(c == 0), stop=(c == HC - 1))
                nc.vector.tensor_scalar_add(
                    dst_c[:, qc * 512:(qc + 1) * 512], ps2[:, :], bias_c[w.replace("w", "b")][:])

        # V^T projection then PE-transpose into v_sb [k, d] blocks
        for qc in range(NQ):   # qc = chunk of 512 key positions
            ps = psA.tile([128, 512], F32, tag="proj", bufs=2)
            for c in range(HC):
                nc.tensor.matmul(ps[:, :],
                                 lhsT=wT["wv"][:, c, 0:128],
                                 rhs=cxT[:, c, qc * 512:(qc + 1) * 512],
                                 start=(c == 0), stop=(c == HC - 1))
            vt_ab = tmpA.tile([128, 512], F32, tag="vt_ab", bufs=2)
            nc.vector.tensor_scalar_add(vt_ab[:, :], ps[:, :], bias_ab["bv"][:])
            ps2 = psA.tile([64, 512], F32, tag="projc", bufs=2)
            for c in range(HC):
                nc.tensor.matmul(ps2[:, :],
                                 lhsT=wT["wv"][:, c, 128:RPC],
                                 rhs=cxT[:, c, qc * 512:(qc + 1) * 512],
                                 start=(c == 0), stop=(c == HC - 1))
            vt_c = tmpA.tile([64, 512], F32, tag="vt_c", bufs=2)
            nc.vector.tensor_scalar_add(vt_c[:, :], ps2[:, :], bias_c["bv"][:])

            pst = psA.tile([128, 512], F32, tag="vtr", bufs=2)
            for j in range(4):
                nc.tensor.transpose(pst[:, j * 128:(j + 1) * 128],
                                    vt_ab[:, j * 128:(j + 1) * 128], ident[:, :])
            # pst[:, j*128:...] = [k(128), d(128)] for k-chunk 4*qc+j, heads 0|1
            dst_ab = v_sb[:, qc * 4:(qc + 1) * 4, 0:130].rearrange(
                "p j (t x) -> p j t x", t=2)[:, :, :, 0:64]
            nc.vector.tensor_copy(
                dst_ab, pst[:, :].rearrange("p (j t x) -> p j t x", j=4, t=2))

            pstc = psA.tile([128, 256], F32, tag="vtrc", bufs=2)
            for j in range(4):
                nc.tensor.transpose(pstc[:, j * 64:(j + 1) * 64],
                                    vt_c[:, j * 128:(j + 1) * 128], ident[0:64, 0:64])
            nc.vector.tensor_copy(
                v_sb[:, qc * 4:(qc + 1) * 4, 130:194],
                pstc[:, :].rearrange("p (j x) -> p j x", j=4))

    # ---- phase B: attention --------------------------------------------
    def attention_pass(psB, tmpB, heads, q0, qw):
        """heads: list of (name, qt_ap, kt_ap, pbase, head_idx)."""
        nq = qw // 512
        ctx_ps = {}
        for name, qt, kt, pb, h in heads:
            ctx_ps[h] = psB.tile([65, qw], F32, tag=f"ctx{name}", name=f"ctx{name}")
        for kc in range(KC):
            exs = {}
            for name, qt, kt, pb, h in heads:
                sp = psB.tile([128, qw], F32, tag=f"s{name}", name=f"s{name}")
                for qi in range(nq):
                    nc.tensor.matmul(
                        sp[:, qi * 512:(qi + 1) * 512],
                        lhsT=kt[pb:pb + 64, kc * 128:(kc + 1) * 128],
                        rhs=qt[pb:pb + 64, q0 + qi * 512:q0 + (qi + 1) * 512],
                        start=True, stop=True)
                ex = tmpB.tile([128, qw], F32R, tag=f"e{name}", name=f"e{name}", bufs=2)
                nc.scalar.activation(ex[:, :], sp[:, :], EXP,
                                     bias=mask_sb[:, kc:kc + 1], scale=SCALE)
                exs[h] = ex
            for name, qt, kt, pb, h in heads:
                for qi in range(nq):
                    nc.tensor.matmul(
                        ctx_ps[h][:, qi * 512:(qi + 1) * 512],
                        lhsT=v_sb[:, kc, h * 65:h * 65 + 65],
                        rhs=exs[h][:, qi * 512:(qi + 1) * 512],
                        start=(kc == 0), stop=(kc == KC - 1))
        # epilogue: normalize + transpose to [q, d]
        for name, qt, kt, pb, h in heads:
            cs = tmpB.tile([65, qw], F32, tag="cs", name=f"cs{name}", bufs=2)
            nc.vector.tensor_copy(cs[:, :], ctx_ps[h][:, :])
            for j in range(qw // 128):
                jj = q0 // 128 + j
                tp = psB.tile([128, 65], F32, tag=f"s{name}", name=f"tp{name}")
                nc.tensor.transpose(tp[:, 0:65], cs[:, j * 128:(j + 1) * 128],
                                    ident[0:65, 0:65])
                rd = tmpB.tile([128, 1], F32, tag="rd", name=f"rd{name}", bufs=4)
                nc.vector.reciprocal(rd[:, :], tp[:, 64:65])
                nc.vector.tensor_scalar_mul(
                    out_sb[:, jj, h * 64:(h + 1) * 64], tp[:, 0:64], rd[:, :])

    with tc.tile_pool(name="tmpB1", bufs=1) as tmpB1, \
         tc.tile_pool(name="psB1", space="PSUM", bufs=1) as psB1:
        for qh in range(2):
            attention_pass(psB1, tmpB1,
                           [("a", qt_ab, kt_ab, 0, 0), ("b", qt_ab, kt_ab, 64, 1)],
                           q0=qh * 1024, qw=1024)
    with tc.tile_pool(name="tmpB2", bufs=1) as tmpB2, \
         tc.tile_pool(name="psB2", space="PSUM", bufs=1) as psB2:
        attention_pass(psB2, tmpB2, [("c", qt_c, kt_c, 0, 2)], q0=0, qw=2048)

    for sc in range(KC):
        nc.sync.dma_start(d["out"][sc * 128:(sc + 1) * 128, :], out_sb[:, sc, :])

    outp.release()
    qkv.release()
    cst.release()


def _build():
    nc = bacc.Bacc("TRN2", target_bir_lowering=False, debug=False,
                   num_devices=NCORES)
    d = {}
    def dram(name, shape, out=False, dt=F32):
        d[name] = nc.dram_tensor(
            name, shape, dt,
            kind="ExternalOutput" if out else "ExternalInput").ap()
    dram("hsT", [HIDDEN, S], dt=F32R)
    dram("ctxT", [HIDDEN, S], dt=F32R)
    dram("wqT", [HIDDEN, RPC], dt=F32R)
    dram("wkT", [HIDDEN, RPC], dt=F32R)
    dram("wvT", [HIDDEN, RPC], dt=F32R)
    dram("bq", [RPC, 1])
    dram("bk", [RPC, 1])
    dram("bv", [RPC, 1])
    dram("mask", [128, KC])
    dram("ones", [128, KC * 3], dt=F32R)
    dram("out", [S, RPC], out=True)
    with tile.TileContext(nc) as tc:
        _emit(tc, nc, d)
    nc.compile()
    return nc


def _get_nc():
    global _cached_nc
    if _cached_nc is None:
        _cached_nc = _build()
    return _cached_nc


def make_in_maps(hidden_states, context, attention_mask, Wq, bq, Wk, bk, Wv, bv):
    f = lambda a: np.ascontiguousarray(np.asarray(a, dtype=np.float32))
    hs, cx, mask = f(hidden_states), f(context), f(attention_mask)
    Wq, Wk, Wv = f(Wq), f(Wk), f(Wv)
    bq, bk, bv = f(bq), f(bk), f(bv)
    in_maps = []
    for core in range(NCORES):
        b = core // 4
        g = core % 4
        rows = slice(g * RPC, (g + 1) * RPC)
        in_maps.append({
            "hsT": np.ascontiguousarray(hs[b].T),
            "ctxT": np.ascontiguousarray(cx[b].T),
            "wqT": np.ascontiguousarray(Wq[rows].T),
            "wkT": np.ascontiguousarray(Wk[rows].T),
            "wvT": np.ascontiguousarray(Wv[rows].T),
            "bq": np.ascontiguousarray(bq[rows].reshape(RPC, 1)),
            "bk": np.ascontiguousarray(bk[rows].reshape(RPC, 1)),
            "bv": np.ascontiguousarray(bv[rows].reshape(RPC, 1)),
            "mask": np.ascontiguousarray(mask[b, 0, 0, :].reshape(KC, 128).T),
            "ones": np.ones((128, KC * 3), dtype=np.float32),
        })
    return in_maps


def gather_out(results):
    outs = [results[i]["out"] for i in range(NCORES)]
    return np.stack([np.concatenate([outs[b * 4 + g] for g in range(4)], axis=1)
                     for b in range(B)]).astype(np.float32)


def kernel(hidden_states, context, attention_mask, Wq, bq, Wk, bk, Wv, bv,
           trace=False):
    nc = _get_nc()
    in_maps = make_in_maps(hidden_states, context, attention_mask,
                           Wq, bq, Wk, bk, Wv, bv)
    res = run_bass_kernel_spmd(nc, in_maps, core_ids=list(range(NCORES)),
                               trace=trace)
    out = gather_out(res.results)
    if trace:
        kernel.last_results = res
    return out


# revision 7
# speedup vs baseline: 1.1139x; 1.1139x over previous
"""Trainium2 Bass kernel for BertAttention (cross-attention).

B=2, S=2048, HIDDEN=768, 12 heads x head_dim 64, float32.

Sharding: (batch, head) pairs = 2*12 = 24 units over 8 cores -> each core
handles batch (core//4) and 3 heads (3*(core%4) .. +2). Attention is
head-local so there is no cross-core traffic.

Per-core pipeline (everything f32; matmuls run in float32r mode):
  - host pre-transposes activations/weights so the device contracts the
    hidden dim directly (hsT/ctxT: [768, 2048], wT: [768, 192])
  - Q^T, K^T computed in [d, seq] layout, 2 heads packed per 128
    partitions; V^T computed then PE-transposed to V [k, d] with a ones
    column appended (-> PV matmul also produces the softmax denominator)
  - attention without max-subtraction: S^T[k, q] per 128-k-chunk,
    ACT exp(scale*x + mask[k]) with the mask folded in as per-partition
    bias, PV accumulates ctx^T (+ denominator) in PSUM over k-chunks
  - epilogue: PE-transpose ctx^T -> ctx[q, d], DVE reciprocal +
    per-partition scale by 1/denominator, DMA out.
"""

import numpy as np

import concourse.bass as bass
import concourse.mybir as mybir
import concourse.tile as tile
from concourse import bacc
from concourse.bass_utils import run_bass_kernel_spmd
from concourse.masks import make_identity

S = 2048
HIDDEN = 768
D = 64
B = 2
NCORES = 8
HPC = 3                  # heads per core
RPC = HPC * D            # 192 weight rows / output cols per core
HC = HIDDEN // 128       # 6 hidden chunks
KC = S // 128            # 16 key chunks
NQ = S // 512            # 4 query chunks of 512
SCALE = float(1.0 / np.sqrt(D + 1e-5))
F32 = mybir.dt.float32
F32R = mybir.dt.float32r
EXP = mybir.ActivationFunctionType.Exp

_cached_nc = None


def _emit(tc, nc, d):
    """Emit the whole per-core program. d maps dram tensor names -> APs."""
    cst = tc.alloc_tile_pool(name="cst", bufs=1)
    qkv = tc.alloc_tile_pool(name="qkv", bufs=1)
    outp = tc.alloc_tile_pool(name="outp", bufs=1)

    ident = cst.tile([128, 128], F32, tag="ident")
    make_identity(nc, ident[:])
    mask_sb = cst.tile([128, KC], F32, tag="mask")
    nc.sync.dma_start(mask_sb[:], d["mask"][:])
    bias_ab = {}
    bias_c = {}
    for w in ("bq", "bk", "bv"):
        bias_ab[w] = cst.tile([128, 1], F32, tag=f"{w}ab", name=f"{w}ab")
        nc.sync.dma_start(bias_ab[w][:], d[w][0:128, :])
        bias_c[w] = cst.tile([64, 1], F32, tag=f"{w}c", name=f"{w}c")
        nc.sync.dma_start(bias_c[w][:], d[w][128:RPC, :])

    # persistent activation tensors
    qt_ab = qkv.tile([128, S], F32R, tag="qt_ab")   # Q^T heads 0,1 (d on partitions)
    qt_c = qkv.tile([64, S], F32R, tag="qt_c")      # Q^T head 2
    kt_ab = qkv.tile([128, S], F32R, tag="kt_ab")
    kt_c = qkv.tile([64, S], F32R, tag="kt_c")
    # V in [k, d] layout: per k-chunk 3 blocks of 65 (64 dims + ones col)
    v_sb = qkv.tile([128, KC, 3 * 65], F32R, tag="v_sb")
    ones_dst = v_sb[:, :, 0:195].rearrange("p k (h x) -> p k h x", h=3)[:, :, :, 64:65]
    nc.sync.dma_start(ones_dst, d["ones"][:, :].rearrange("p (k h x) -> p k h x", k=KC, h=3))

    out_sb = outp.tile([128, KC, RPC], F32, tag="out_sb")

    # ---- phase A: load + QKV projections -------------------------------
    with tc.tile_pool(name="acts", bufs=1) as acts, \
         tc.tile_pool(name="wts", bufs=1) as wts, \
         tc.tile_pool(name="tmpA", bufs=1) as tmpA, \
         tc.tile_pool(name="psA", space="PSUM", bufs=1) as psA:

        hsT = acts.tile([128, HC, S], F32R, tag="hsT")
        cxT = acts.tile([128, HC, S], F32R, tag="cxT")
        wT = {}
        for w in ("wq", "wk", "wv"):
            wT[w] = wts.tile([128, HC, RPC], F32R, tag=f"{w}T", name=f"{w}T")
            for c in range(HC):
                nc.sync.dma_start(wT[w][:, c, :],
                                  d[w + "T"][c * 128:(c + 1) * 128, :])
        for c in range(HC):
            nc.sync.dma_start(cxT[:, c, :], d["ctxT"][c * 128:(c + 1) * 128, :])
        for c in range(HC):
            nc.sync.dma_start(hsT[:, c, :], d["hsT"][c * 128:(c + 1) * 128, :])

        # K^T / Q^T projections: out[d, q] = sum_hid W^T[hid, d]^T actT[hid, q]
        for w, rhs, dst_ab, dst_c in (("wk", cxT, kt_ab, kt_c),
                                      ("wq", hsT, qt_ab, qt_c)):
            for qc in range(NQ):
                ps = psA.tile([128, 512], F32, tag="proj", bufs=2)
                for c in range(HC):
                    nc.tensor.matmul(ps[:, :],
                                     lhsT=wT[w][:, c, 0:128],
                                     rhs=rhs[:, c, qc * 512:(qc + 1) * 512],
                                     start=(c == 0), stop=(c == HC - 1))
                nc.vector.tensor_scalar_add(
                    dst_ab[:, qc * 512:(qc + 1) * 512], ps[:, :], bias_ab[w.replace("w", "b")][:])
                ps2 = psA.tile([64, 512], F32, tag="projc", bufs=2)
                for c in range(HC):
                    nc.tensor.matmul(ps2[:, :],
                                     lhsT=wT[w][:, c, 128:RPC],
                                     rhs=rhs[:, c, qc * 512:(qc + 1) * 512],
                                     start=(c == 0), stop=(c == HC - 1))
                nc.vector.tensor_scalar_add(
                    dst_c[:, qc * 512:(qc + 1) * 512], ps2[:, :], bias_c[w.replace("w", "b")][:])

        # V^T projection then PE-transpose into v_sb [k, d] blocks
        for qc in range(NQ):   # qc = chunk of 512 key positions
            ps = psA.tile([128, 512], F32, tag="proj", bufs=2)
            for c in range(HC):
                nc.tensor.matmul(ps[:, :],
                                 lhsT=wT["wv"][:, c, 0:128],
                                 rhs=cxT[:, c, qc * 512:(qc + 1) * 512],
                                 start=(c == 0), stop=(c == HC - 1))
            vt_ab = tmpA.tile([128, 512], F32, tag="vt_ab", bufs=2)
            nc.vector.tensor_scalar_add(vt_ab[:, :], ps[:, :], bias_ab["bv"][:])
            ps2 = psA.tile([64, 512], F32, tag="projc", bufs=2)
            for c in range(HC):
                nc.tensor.matmul(ps2[:, :],
                                 lhsT=wT["wv"][:, c, 128:RPC],
                                 rhs=cxT[:, c, qc * 512:(qc + 1) * 512],
                                 start=(c == 0), stop=(c == HC - 1))
            vt_c = tmpA.tile([64, 512], F32, tag="vt_c", bufs=2)
            nc.vector.tensor_scalar_add(vt_c[:, :], ps2[:, :], bias_c["bv"][:])

            pst = psA.tile([128, 512], F32, tag="vtr", bufs=2)
            for j in range(4):
                nc.tensor.transpose(pst[:, j * 128:(j + 1) * 128],
                                    vt_ab[:, j * 128:(j + 1) * 128], ident[:, :])
            # pst[:, j*128:...] = [k(128), d(128)] for k-chunk 4*qc+j, heads 0|1
            dst_ab = v_sb[:, qc * 4:(qc + 1) * 4, 0:130].rearrange(
                "p j (t x) -> p j t x", t=2)[:, :, :, 0:64]
            nc.vector.tensor_copy(
                dst_ab, pst[:, :].rearrange("p (j t x) -> p j t x", j=4, t=2))

            pstc = psA.tile([128, 256], F32, tag="vtrc", bufs=2)
            for j in range(4):
                nc.tensor.transpose(pstc[:, j * 64:(j + 1) * 64],
                                    vt_c[:, j * 128:(j + 1) * 128], ident[0:64, 0:64])
            nc.vector.tensor_copy(
                v_sb[:, qc * 4:(qc + 1) * 4, 130:194],
                pstc[:, :].rearrange("p (j x) -> p j x", j=4))

    # ---- phase B: attention --------------------------------------------
    # One (head, query-half) unit at a time; kc-pipelined scores -> exp ->
    # PV chain with double-buffered PSUM so consecutive units overlap and
    # the PE never idles long enough to drop its clock.
    units = []
    for qh in range(2):
        units.append(("a", qt_ab, kt_ab, 0, 0, qh))
        units.append(("b", qt_ab, kt_ab, 64, 1, qh))
        units.append(("c", qt_c, kt_c, 0, 2, qh))

    with tc.tile_pool(name="tmpB", bufs=1) as tmpB, \
         tc.tile_pool(name="psB", space="PSUM", bufs=1) as psB:
        for name, qt, kt, pb, h, qh in units:
            q0 = qh * 1024
            ctx_ps = psB.tile([65, 1024], F32, tag="ctx", bufs=2,
                              name=f"ctx{name}{qh}")
            for kc in range(KC):
                sp = psB.tile([128, 1024], F32, tag="s", bufs=2,
                              name=f"s{name}{qh}")
                for qi in range(2):
                    nc.tensor.matmul(
                        sp[:, qi * 512:(qi + 1) * 512],
                        lhsT=kt[pb:pb + 64, kc * 128:(kc + 1) * 128],
                        rhs=qt[pb:pb + 64, q0 + qi * 512:q0 + (qi + 1) * 512],
                        start=True, stop=True)
                ex = tmpB.tile([128, 1024], F32R, tag="e", bufs=6,
                               name=f"e{name}{qh}")
                nc.scalar.activation(ex[:, :], sp[:, :], EXP,
                                     bias=mask_sb[:, kc:kc + 1], scale=SCALE)
                for qi in range(2):
                    nc.tensor.matmul(
                        ctx_ps[:, qi * 512:(qi + 1) * 512],
                        lhsT=v_sb[:, kc, h * 65:h * 65 + 65],
                        rhs=ex[:, qi * 512:(qi + 1) * 512],
                        start=(kc == 0), stop=(kc == KC - 1))
            # epilogue: normalize + transpose to [q, d]
            cs = tmpB.tile([65, 1024], F32, tag="cs", bufs=2, name=f"cs{name}{qh}")
            nc.vector.tensor_copy(cs[:, :], ctx_ps[:, :])
            for half in range(2):
                tp = psB.tile([128, 4, 65], F32, tag="s", bufs=2, name=f"tp{name}{qh}")
                for j4 in range(4):
                    j = half * 4 + j4
                    nc.tensor.transpose(tp[:, j4, :], cs[:, j * 128:(j + 1) * 128],
                                        ident[0:65, 0:65])
                for j4 in range(4):
                    jj = qh * 8 + half * 4 + j4
                    rd = tmpB.tile([128, 1], F32, tag="rd", bufs=4,
                                   name=f"rd{name}{qh}")
                    nc.vector.reciprocal(rd[:, :], tp[:, j4, 64:65])
                    nc.vector.tensor_scalar_mul(
                        out_sb[:, jj, h * 64:(h + 1) * 64], tp[:, j4, 0:64],
                        rd[:, :])

    for sc in range(KC):
        nc.sync.dma_start(d["out"][sc * 128:(sc + 1) * 128, :], out_sb[:, sc, :])

    outp.release()
    qkv.release()
    cst.release()


def _build():
    nc = bacc.Bacc("TRN2", target_bir_lowering=False, debug=False,
                   num_devices=NCORES)
    d = {}
    def dram(name, shape, out=False, dt=F32):
        d[name] = nc.dram_tensor(
            name, shape, dt,
            kind="ExternalOutput" if out else "ExternalInput").ap()
    dram("hsT", [HIDDEN, S], dt=F32R)
    dram("ctxT", [HIDDEN, S], dt=F32R)
    dram("wqT", [HIDDEN, RPC], dt=F32R)
    dram("wkT", [HIDDEN, RPC], dt=F32R)
    dram("wvT", [HIDDEN, RPC], dt=F32R)
    dram("bq", [RPC, 1])
    dram("bk", [RPC, 1])
    dram("bv", [RPC, 1])
    dram("mask", [128, KC])
    dram("ones", [128, KC * 3], dt=F32R)
    dram("out", [S, RPC], out=True)
    with tile.TileContext(nc) as tc:
        _emit(tc, nc, d)
    nc.compile()
    return nc


def _get_nc():
    global _cached_nc
    if _cached_nc is None:
        _cached_nc = _build()
    return _cached_nc


def make_in_maps(hidden_states, context, attention_mask, Wq, bq, Wk, bk, Wv, bv):
    f = lambda a: np.ascontiguousarray(np.asarray(a, dtype=np.float32))
    hs, cx, mask = f(hidden_states), f(context), f(attention_mask)
    Wq, Wk, Wv = f(Wq), f(Wk), f(Wv)
    bq, bk, bv = f(bq), f(bk), f(bv)
    in_maps = []
    for core in range(NCORES):
        b = core // 4
        g = core % 4
        rows = slice(g * RPC, (g + 1) * RPC)
        in_maps.append({
            "hsT": np.ascontiguousarray(hs[b].T),
            "ctxT": np.ascontiguousarray(cx[b].T),
            "wqT": np.ascontiguousarray(Wq[rows].T),
            "wkT": np.ascontiguousarray(Wk[rows].T),
            "wvT": np.ascontiguousarray(Wv[rows].T),
            "bq": np.ascontiguousarray(bq[rows].reshape(RPC, 1)),
            "bk": np.ascontiguousarray(bk[rows].reshape(RPC, 1)),
            "bv": np.ascontiguousarray(bv[rows].reshape(RPC, 1)),
            "mask": np.ascontiguousarray(mask[b, 0, 0, :].reshape(KC, 128).T),
            "ones": np.ones((128, KC * 3), dtype=np.float32),
        })
    return in_maps


def gather_out(results):
    outs = [results[i]["out"] for i in range(NCORES)]
    return np.stack([np.concatenate([outs[b * 4 + g] for g in range(4)], axis=1)
                     for b in range(B)]).astype(np.float32)


def kernel(hidden_states, context, attention_mask, Wq, bq, Wk, bk, Wv, bv,
           trace=False):
    nc = _get_nc()
    in_maps = make_in_maps(hidden_states, context, attention_mask,
                           Wq, bq, Wk, bk, Wv, bv)
    res = run_bass_kernel_spmd(nc, in_maps, core_ids=list(range(NCORES)),
                               trace=trace)
    out = gather_out(res.results)
    if trace:
        kernel.last_results = res
    return out


# revision 8
# speedup vs baseline: 1.1903x; 1.0686x over previous
"""Trainium2 Bass kernel for BertAttention (cross-attention).

B=2, S=2048, HIDDEN=768, 12 heads x head_dim 64, float32.

Sharding: (batch, head) pairs = 2*12 = 24 units over 8 cores -> each core
handles batch (core//4) and 3 heads (3*(core%4) .. +2). Attention is
head-local so there is no cross-core traffic.

Per-core pipeline (everything f32; matmuls run in float32r mode):
  - host pre-transposes activations/weights so the device contracts the
    hidden dim directly (hsT/ctxT: [768, 2048], wT: [768, 192])
  - Q^T, K^T computed in [d, seq] layout, 2 heads packed per 128
    partitions; V^T computed then PE-transposed to V [k, d] with a ones
    column appended (-> PV matmul also produces the softmax denominator)
  - attention without max-subtraction: S^T[k, q] per 128-k-chunk,
    ACT exp(scale*x + mask[k]) with the mask folded in as per-partition
    bias, PV accumulates ctx^T (+ denominator) in PSUM over k-chunks
  - epilogue: PE-transpose ctx^T -> ctx[q, d], DVE reciprocal +
    per-partition scale by 1/denominator, DMA out.
"""

import ml_dtypes
import numpy as np

import concourse.bass as bass
import concourse.mybir as mybir
import concourse.tile as tile
from concourse import bacc
from concourse.bass_utils import run_bass_kernel_spmd
from concourse.masks import make_identity

S = 2048
HIDDEN = 768
D = 64
B = 2
NCORES = 8
HPC = 3                  # heads per core
RPC = HPC * D            # 192 weight rows / output cols per core
HC = HIDDEN // 128       # 6 hidden chunks
KC = S // 128            # 16 key chunks
NQ = S // 512            # 4 query chunks of 512
SCALE = float(1.0 / np.sqrt(D + 1e-5))
F32 = mybir.dt.float32
F32R = mybir.dt.float32r
BF16 = mybir.dt.bfloat16
EXP = mybir.ActivationFunctionType.Exp
_bf16np = ml_dtypes.bfloat16

_cached_nc = None


def _emit(tc, nc, d):
    """Emit the whole per-core program. d maps dram tensor names -> APs."""
    cst = tc.alloc_tile_pool(name="cst", bufs=1)
    qkv = tc.alloc_tile_pool(name="qkv", bufs=1)
    outp = tc.alloc_tile_pool(name="outp", bufs=1)

    ident = cst.tile([128, 128], F32, tag="ident")
    make_identity(nc, ident[:])
    mask_sb = cst.tile([128, KC], F32, tag="mask")
    nc.sync.dma_start(mask_sb[:], d["mask"][:])
    bias_ab = {}
    bias_c = {}
    for w in ("bq", "bk", "bv"):
        bias_ab[w] = cst.tile([128, 1], F32, tag=f"{w}ab", name=f"{w}ab")
        nc.sync.dma_start(bias_ab[w][:], d[w][0:128, :])
        bias_c[w] = cst.tile([64, 1], F32, tag=f"{w}c", name=f"{w}c")
        nc.sync.dma_start(bias_c[w][:], d[w][128:RPC, :])

    # persistent activation tensors
    qt_ab = qkv.tile([128, S], BF16, tag="qt_ab")   # Q^T heads 0,1 (d on partitions)
    qt_c = qkv.tile([64, S], BF16, tag="qt_c")      # Q^T head 2
    kt_ab = qkv.tile([128, S], BF16, tag="kt_ab")
    kt_c = qkv.tile([64, S], BF16, tag="kt_c")
    # V in [k, d] layout: per k-chunk 3 blocks of 65 (64 dims + ones col)
    v_sb = qkv.tile([128, KC, 3 * 65], BF16, tag="v_sb")
    ones_dst = v_sb[:, :, 0:195].rearrange("p k (h x) -> p k h x", h=3)[:, :, :, 64:65]
    nc.sync.dma_start(ones_dst, d["ones"][:, :].rearrange("p (k h x) -> p k h x", k=KC, h=3))

    out_sb = outp.tile([128, KC, RPC], F32, tag="out_sb")

    # ---- phase A: load + QKV projections -------------------------------
    with tc.tile_pool(name="acts", bufs=1) as acts, \
         tc.tile_pool(name="wts", bufs=1) as wts, \
         tc.tile_pool(name="tmpA", bufs=1) as tmpA, \
         tc.tile_pool(name="psA", space="PSUM", bufs=1) as psA:

        hsT = acts.tile([128, HC, S], F32R, tag="hsT")
        cxT = acts.tile([128, HC, S], F32R, tag="cxT")
        wT = {}
        for w in ("wq", "wk", "wv"):
            wT[w] = wts.tile([128, HC, RPC], F32R, tag=f"{w}T", name=f"{w}T")
            for c in range(HC):
                nc.sync.dma_start(wT[w][:, c, :],
                                  d[w + "T"][c * 128:(c + 1) * 128, :])
        for c in range(HC):
            nc.sync.dma_start(cxT[:, c, :], d["ctxT"][c * 128:(c + 1) * 128, :])
        for c in range(HC):
            nc.sync.dma_start(hsT[:, c, :], d["hsT"][c * 128:(c + 1) * 128, :])

        # K^T / Q^T projections: out[d, q] = sum_hid W^T[hid, d]^T actT[hid, q]
        for w, rhs, dst_ab, dst_c in (("wk", cxT, kt_ab, kt_c),
                                      ("wq", hsT, qt_ab, qt_c)):
            for qc in range(NQ):
                ps = psA.tile([128, 512], F32, tag="proj", bufs=2)
                for c in range(HC):
                    nc.tensor.matmul(ps[:, :],
                                     lhsT=wT[w][:, c, 0:128],
                                     rhs=rhs[:, c, qc * 512:(qc + 1) * 512],
                                     start=(c == 0), stop=(c == HC - 1))
                nc.vector.tensor_scalar_add(
                    dst_ab[:, qc * 512:(qc + 1) * 512], ps[:, :], bias_ab[w.replace("w", "b")][:])
                ps2 = psA.tile([64, 512], F32, tag="projc", bufs=2)
                for c in range(HC):
                    nc.tensor.matmul(ps2[:, :],
                                     lhsT=wT[w][:, c, 128:RPC],
                                     rhs=rhs[:, c, qc * 512:(qc + 1) * 512],
                                     start=(c == 0), stop=(c == HC - 1))
                nc.vector.tensor_scalar_add(
                    dst_c[:, qc * 512:(qc + 1) * 512], ps2[:, :], bias_c[w.replace("w", "b")][:])

        # V^T projection then PE-transpose into v_sb [k, d] blocks
        for qc in range(NQ):   # qc = chunk of 512 key positions
            ps = psA.tile([128, 512], F32, tag="proj", bufs=2)
            for c in range(HC):
                nc.tensor.matmul(ps[:, :],
                                 lhsT=wT["wv"][:, c, 0:128],
                                 rhs=cxT[:, c, qc * 512:(qc + 1) * 512],
                                 start=(c == 0), stop=(c == HC - 1))
            vt_ab = tmpA.tile([128, 512], F32, tag="vt_ab", bufs=2)
            nc.vector.tensor_scalar_add(vt_ab[:, :], ps[:, :], bias_ab["bv"][:])
            ps2 = psA.tile([64, 512], F32, tag="projc", bufs=2)
            for c in range(HC):
                nc.tensor.matmul(ps2[:, :],
                                 lhsT=wT["wv"][:, c, 128:RPC],
                                 rhs=cxT[:, c, qc * 512:(qc + 1) * 512],
                                 start=(c == 0), stop=(c == HC - 1))
            vt_c = tmpA.tile([64, 512], F32, tag="vt_c", bufs=2)
            nc.vector.tensor_scalar_add(vt_c[:, :], ps2[:, :], bias_c["bv"][:])

            pst = psA.tile([128, 512], F32, tag="vtr", bufs=2)
            for j in range(4):
                nc.tensor.transpose(pst[:, j * 128:(j + 1) * 128],
                                    vt_ab[:, j * 128:(j + 1) * 128], ident[:, :])
            # pst[:, j*128:...] = [k(128), d(128)] for k-chunk 4*qc+j, heads 0|1
            dst_ab = v_sb[:, qc * 4:(qc + 1) * 4, 0:130].rearrange(
                "p j (t x) -> p j t x", t=2)[:, :, :, 0:64]
            nc.vector.tensor_copy(
                dst_ab, pst[:, :].rearrange("p (j t x) -> p j t x", j=4, t=2))

            pstc = psA.tile([128, 256], F32, tag="vtrc", bufs=2)
            for j in range(4):
                nc.tensor.transpose(pstc[:, j * 64:(j + 1) * 64],
                                    vt_c[:, j * 128:(j + 1) * 128], ident[0:64, 0:64])
            nc.vector.tensor_copy(
                v_sb[:, qc * 4:(qc + 1) * 4, 130:194],
                pstc[:, :].rearrange("p (j x) -> p j x", j=4))

    # ---- phase B: attention --------------------------------------------
    # One (head, query-half) unit at a time; kc-pipelined scores -> exp ->
    # PV chain with double-buffered PSUM so consecutive units overlap and
    # the PE never idles long enough to drop its clock.
    units = []
    for qh in range(2):
        units.append(("a", qt_ab, kt_ab, 0, 0, qh))
        units.append(("b", qt_ab, kt_ab, 64, 1, qh))
        units.append(("c", qt_c, kt_c, 0, 2, qh))

    with tc.tile_pool(name="tmpB", bufs=1) as tmpB, \
         tc.tile_pool(name="psB", space="PSUM", bufs=1) as psB:
        for name, qt, kt, pb, h, qh in units:
            q0 = qh * 1024
            ctx_ps = psB.tile([65, 1024], F32, tag="ctx", bufs=2,
                              name=f"ctx{name}{qh}")
            for kc in range(KC):
                sp = psB.tile([128, 1024], F32, tag="s", bufs=2,
                              name=f"s{name}{qh}")
                for qi in range(2):
                    nc.tensor.matmul(
                        sp[:, qi * 512:(qi + 1) * 512],
                        lhsT=kt[pb:pb + 64, kc * 128:(kc + 1) * 128],
                        rhs=qt[pb:pb + 64, q0 + qi * 512:q0 + (qi + 1) * 512],
                        start=True, stop=True)
                ex = tmpB.tile([128, 1024], BF16, tag="e", bufs=6,
                               name=f"e{name}{qh}")
                nc.scalar.activation(ex[:, :], sp[:, :], EXP,
                                     bias=mask_sb[:, kc:kc + 1], scale=SCALE)
                for qi in range(2):
                    nc.tensor.matmul(
                        ctx_ps[:, qi * 512:(qi + 1) * 512],
                        lhsT=v_sb[:, kc, h * 65:h * 65 + 65],
                        rhs=ex[:, qi * 512:(qi + 1) * 512],
                        start=(kc == 0), stop=(kc == KC - 1))
            # epilogue: normalize + transpose to [q, d]
            cs = tmpB.tile([65, 1024], F32, tag="cs", bufs=2, name=f"cs{name}{qh}")
            nc.vector.tensor_copy(cs[:, :], ctx_ps[:, :])
            for half in range(2):
                tp = psB.tile([128, 4, 65], F32, tag="s", bufs=2, name=f"tp{name}{qh}")
                for j4 in range(4):
                    j = half * 4 + j4
                    nc.tensor.transpose(tp[:, j4, :], cs[:, j * 128:(j + 1) * 128],
                                        ident[0:65, 0:65])
                for j4 in range(4):
                    jj = qh * 8 + half * 4 + j4
                    rd = tmpB.tile([128, 1], F32, tag="rd", bufs=4,
                                   name=f"rd{name}{qh}")
                    nc.vector.reciprocal(rd[:, :], tp[:, j4, 64:65])
                    nc.vector.tensor_scalar_mul(
                        out_sb[:, jj, h * 64:(h + 1) * 64], tp[:, j4, 0:64],
                        rd[:, :])

    for sc in range(KC):
        nc.sync.dma_start(d["out"][sc * 128:(sc + 1) * 128, :], out_sb[:, sc, :])

    outp.release()
    qkv.release()
    cst.release()


def _build():
    nc = bacc.Bacc("TRN2", target_bir_lowering=False, debug=False,
                   num_devices=NCORES)
    d = {}
    def dram(name, shape, out=False, dt=F32):
        d[name] = nc.dram_tensor(
            name, shape, dt,
            kind="ExternalOutput" if out else "ExternalInput").ap()
    dram("hsT", [HIDDEN, S], dt=F32R)
    dram("ctxT", [HIDDEN, S], dt=F32R)
    dram("wqT", [HIDDEN, RPC], dt=F32R)
    dram("wkT", [HIDDEN, RPC], dt=F32R)
    dram("wvT", [HIDDEN, RPC], dt=F32R)
    dram("bq", [RPC, 1])
    dram("bk", [RPC, 1])
    dram("bv", [RPC, 1])
    dram("mask", [128, KC])
    dram("ones", [128, KC * 3], dt=BF16)
    dram("out", [S, RPC], out=True)
    with tile.TileContext(nc) as tc:
        _emit(tc, nc, d)
    nc.compile()
    return nc


def _get_nc():
    global _cached_nc
    if _cached_nc is None:
        _cached_nc = _build()
    return _cached_nc


def make_in_maps(hidden_states, context, attention_mask, Wq, bq, Wk, bk, Wv, bv):
    f = lambda a: np.ascontiguousarray(np.asarray(a, dtype=np.float32))
    hs, cx, mask = f(hidden_states), f(context), f(attention_mask)
    Wq, Wk, Wv = f(Wq), f(Wk), f(Wv)
    bq, bk, bv = f(bq), f(bk), f(bv)
    in_maps = []
    for core in range(NCORES):
        b = core // 4
        g = core % 4
        rows = slice(g * RPC, (g + 1) * RPC)
        in_maps.append({
            "hsT": np.ascontiguousarray(hs[b].T),
            "ctxT": np.ascontiguousarray(cx[b].T),
            "wqT": np.ascontiguousarray(Wq[rows].T),
            "wkT": np.ascontiguousarray(Wk[rows].T),
            "wvT": np.ascontiguousarray(Wv[rows].T),
            "bq": np.ascontiguousarray(bq[rows].reshape(RPC, 1)),
            "bk": np.ascontiguousarray(bk[rows].reshape(RPC, 1)),
            "bv": np.ascontiguousarray(bv[rows].reshape(RPC, 1)),
            "mask": np.ascontiguousarray(mask[b, 0, 0, :].reshape(KC, 128).T),
            "ones": np.ones((128, KC * 3), dtype=_bf16np),
        })
    return in_maps


def gather_out(results):
    outs = [results[i]["out"] for i in range(NCORES)]
    return np.stack([np.concatenate([outs[b * 4 + g] for g in range(4)], axis=1)
                     for b in range(B)]).astype(np.float32)


def kernel(hidden_states, context, attention_mask, Wq, bq, Wk, bk, Wv, bv,
           trace=False):
    nc = _get_nc()
    in_maps = make_in_maps(hidden_states, context, attention_mask,
                           Wq, bq, Wk, bk, Wv, bv)
    res = run_bass_kernel_spmd(nc, in_maps, core_ids=list(range(NCORES)),
                               trace=trace)
    out = gather_out(res.results)
    if trace:
        kernel.last_results = res
    return out


# revision 9
# speedup vs baseline: 1.2766x; 1.0725x over previous
"""Trainium2 Bass kernel for BertAttention (cross-attention).

B=2, S=2048, HIDDEN=768, 12 heads x head_dim 64, float32.

Sharding: (batch, head) pairs = 2*12 = 24 units over 8 cores -> each core
handles batch (core//4) and 3 heads (3*(core%4) .. +2). Attention is
head-local so there is no cross-core traffic.

Per-core pipeline (everything f32; matmuls run in float32r mode):
  - host pre-transposes activations/weights so the device contracts the
    hidden dim directly (hsT/ctxT: [768, 2048], wT: [768, 192])
  - Q^T, K^T computed in [d, seq] layout, 2 heads packed per 128
    partitions; V^T computed then PE-transposed to V [k, d] with a ones
    column appended (-> PV matmul also produces the softmax denominator)
  - attention without max-subtraction: S^T[k, q] per 128-k-chunk,
    ACT exp(scale*x + mask[k]) with the mask folded in as per-partition
    bias, PV accumulates ctx^T (+ denominator) in PSUM over k-chunks
  - epilogue: PE-transpose ctx^T -> ctx[q, d], DVE reciprocal +
    per-partition scale by 1/denominator, DMA out.
"""

import ml_dtypes
import numpy as np

import concourse.bass as bass
import concourse.mybir as mybir
import concourse.tile as tile
from concourse import bacc
from concourse.bass_utils import run_bass_kernel_spmd
from concourse.masks import make_identity

S = 2048
HIDDEN = 768
D = 64
B = 2
NCORES = 8
HPC = 3                  # heads per core
RPC = HPC * D            # 192 weight rows / output cols per core
HC = HIDDEN // 128       # 6 hidden chunks
KC = S // 128            # 16 key chunks
NQ = S // 512            # 4 query chunks of 512
SCALE = float(1.0 / np.sqrt(D + 1e-5))
F32 = mybir.dt.float32
F32R = mybir.dt.float32r
BF16 = mybir.dt.bfloat16
EXP = mybir.ActivationFunctionType.Exp
_bf16np = ml_dtypes.bfloat16

_cached_nc = None


def _emit(tc, nc, d):
    """Emit the whole per-core program. d maps dram tensor names -> APs."""
    cst = tc.alloc_tile_pool(name="cst", bufs=1)
    qkv = tc.alloc_tile_pool(name="qkv", bufs=1)
    outp = tc.alloc_tile_pool(name="outp", bufs=1)

    ident = cst.tile([128, 128], F32, tag="ident")
    make_identity(nc, ident[:])
    mask_sb = cst.tile([128, KC], F32, tag="mask")
    nc.sync.dma_start(mask_sb[:], d["mask"][:])
    bias_ab = {}
    bias_c = {}
    for w in ("bq", "bk", "bv"):
        bias_ab[w] = cst.tile([128, 1], F32, tag=f"{w}ab", name=f"{w}ab")
        nc.sync.dma_start(bias_ab[w][:], d[w][0:128, :])
        bias_c[w] = cst.tile([64, 1], F32, tag=f"{w}c", name=f"{w}c")
        nc.sync.dma_start(bias_c[w][:], d[w][128:RPC, :])

    # persistent activation tensors
    qt_ab = qkv.tile([128, S], BF16, tag="qt_ab")   # Q^T heads 0,1 (d on partitions)
    qt_c = qkv.tile([64, S], BF16, tag="qt_c")      # Q^T head 2
    kt_ab = qkv.tile([128, S], BF16, tag="kt_ab")
    kt_c = qkv.tile([64, S], BF16, tag="kt_c")
    # V in [k, d] layout: per k-chunk 3 blocks of 65 (64 dims + ones col)
    v_sb = qkv.tile([128, KC, 3 * 65], BF16, tag="v_sb")
    ones_dst = v_sb[:, :, 0:195].rearrange("p k (h x) -> p k h x", h=3)[:, :, :, 64:65]
    nc.sync.dma_start(ones_dst, d["ones"][:, :].rearrange("p (k h x) -> p k h x", k=KC, h=3))

    out_sb = outp.tile([128, KC, RPC], F32, tag="out_sb")

    # ---- phase A: load + QKV projections -------------------------------
    with tc.tile_pool(name="acts", bufs=1) as acts, \
         tc.tile_pool(name="wts", bufs=1) as wts, \
         tc.tile_pool(name="tmpA", bufs=1) as tmpA, \
         tc.tile_pool(name="psA", space="PSUM", bufs=1) as psA:

        hsT = acts.tile([128, HC, S], F32R, tag="hsT")
        cxT = acts.tile([128, HC, S], F32R, tag="cxT")
        wT = {}
        for w in ("wq", "wk", "wv"):
            wT[w] = wts.tile([128, HC, RPC], F32R, tag=f"{w}T", name=f"{w}T")
            for c in range(HC):
                nc.sync.dma_start(wT[w][:, c, :],
                                  d[w + "T"][c * 128:(c + 1) * 128, :])
        for c in range(HC):
            nc.sync.dma_start(cxT[:, c, :], d["ctxT"][c * 128:(c + 1) * 128, :])
        for c in range(HC):
            nc.sync.dma_start(hsT[:, c, :], d["hsT"][c * 128:(c + 1) * 128, :])

        # K^T / Q^T projections: out[d, q] = sum_hid W^T[hid, d]^T actT[hid, q]
        for w, rhs, dst_ab, dst_c in (("wk", cxT, kt_ab, kt_c),
                                      ("wq", hsT, qt_ab, qt_c)):
            for qc in range(NQ):
                ps = psA.tile([128, 512], F32, tag="proj", bufs=2)
                for c in range(HC):
                    nc.tensor.matmul(ps[:, :],
                                     lhsT=wT[w][:, c, 0:128],
                                     rhs=rhs[:, c, qc * 512:(qc + 1) * 512],
                                     start=(c == 0), stop=(c == HC - 1))
                nc.vector.tensor_scalar_add(
                    dst_ab[:, qc * 512:(qc + 1) * 512], ps[:, :], bias_ab[w.replace("w", "b")][:])
                ps2 = psA.tile([64, 512], F32, tag="projc", bufs=2)
                for c in range(HC):
                    nc.tensor.matmul(ps2[:, :],
                                     lhsT=wT[w][:, c, 128:RPC],
                                     rhs=rhs[:, c, qc * 512:(qc + 1) * 512],
                                     start=(c == 0), stop=(c == HC - 1))
                nc.vector.tensor_scalar_add(
                    dst_c[:, qc * 512:(qc + 1) * 512], ps2[:, :], bias_c[w.replace("w", "b")][:])

        # V^T projection then PE-transpose into v_sb [k, d] blocks
        for qc in range(NQ):   # qc = chunk of 512 key positions
            ps = psA.tile([128, 512], F32, tag="proj", bufs=2)
            for c in range(HC):
                nc.tensor.matmul(ps[:, :],
                                 lhsT=wT["wv"][:, c, 0:128],
                                 rhs=cxT[:, c, qc * 512:(qc + 1) * 512],
                                 start=(c == 0), stop=(c == HC - 1))
            vt_ab = tmpA.tile([128, 512], F32, tag="vt_ab", bufs=2)
            nc.vector.tensor_scalar_add(vt_ab[:, :], ps[:, :], bias_ab["bv"][:])
            ps2 = psA.tile([64, 512], F32, tag="projc", bufs=2)
            for c in range(HC):
                nc.tensor.matmul(ps2[:, :],
                                 lhsT=wT["wv"][:, c, 128:RPC],
                                 rhs=cxT[:, c, qc * 512:(qc + 1) * 512],
                                 start=(c == 0), stop=(c == HC - 1))
            vt_c = tmpA.tile([64, 512], F32, tag="vt_c", bufs=2)
            nc.vector.tensor_scalar_add(vt_c[:, :], ps2[:, :], bias_c["bv"][:])

            pst = psA.tile([128, 512], F32, tag="vtr", bufs=2)
            for j in range(4):
                nc.tensor.transpose(pst[:, j * 128:(j + 1) * 128],
                                    vt_ab[:, j * 128:(j + 1) * 128], ident[:, :])
            # pst[:, j*128:...] = [k(128), d(128)] for k-chunk 4*qc+j, heads 0|1
            dst_ab = v_sb[:, qc * 4:(qc + 1) * 4, 0:130].rearrange(
                "p j (t x) -> p j t x", t=2)[:, :, :, 0:64]
            nc.vector.tensor_copy(
                dst_ab, pst[:, :].rearrange("p (j t x) -> p j t x", j=4, t=2))

            pstc = psA.tile([128, 256], F32, tag="vtrc", bufs=2)
            for j in range(4):
                nc.tensor.transpose(pstc[:, j * 64:(j + 1) * 64],
                                    vt_c[:, j * 128:(j + 1) * 128], ident[0:64, 0:64])
            nc.vector.tensor_copy(
                v_sb[:, qc * 4:(qc + 1) * 4, 130:194],
                pstc[:, :].rearrange("p (j x) -> p j x", j=4))

    # ---- phase B: attention --------------------------------------------
    # Global software-pipelined stream over (head, query-half, k-chunk)
    # steps. PE program order is sc(i+1) BEFORE pv(i) so the PE never
    # head-of-line blocks behind an exp wait; epilogues drip in 2 steps
    # after their unit finishes.
    units = []
    for qh in range(2):
        units.append(("a", qt_ab, kt_ab, 0, 0, qh))
        units.append(("b", qt_ab, kt_ab, 64, 1, qh))
        units.append(("c", qt_c, kt_c, 0, 2, qh))

    steps = [u + (kc,) for u in units for kc in range(KC)]

    with tc.tile_pool(name="tmpB", bufs=1) as tmpB, \
         tc.tile_pool(name="psB", space="PSUM", bufs=1) as psB:
        ctx_of = {}
        sp_of = {}

        def emit_sc(st):
            name, qt, kt, pb, h, qh, kc = st
            if kc == 0:
                ctx_of[name, qh] = psB.tile([65, 1024], F32, tag="ctx", bufs=2,
                                            name=f"ctx{name}{qh}")
            sp = psB.tile([128, 1024], F32, tag="s", bufs=2, name=f"s{name}{qh}")
            sp_of[name, qh] = sp
            q0 = qh * 1024
            for qi in range(2):
                nc.tensor.matmul(
                    sp[:, qi * 512:(qi + 1) * 512],
                    lhsT=kt[pb:pb + 64, kc * 128:(kc + 1) * 128],
                    rhs=qt[pb:pb + 64, q0 + qi * 512:q0 + (qi + 1) * 512],
                    start=True, stop=True)

        def emit_exp_pv(st, nxt):
            name, qt, kt, pb, h, qh, kc = st
            sp = sp_of[name, qh]
            ex = tmpB.tile([128, 1024], BF16, tag="e", bufs=4, name=f"e{name}{qh}")
            nc.scalar.activation(ex[:, :], sp[:, :], EXP,
                                 bias=mask_sb[:, kc:kc + 1], scale=SCALE)
            if nxt is not None:
                emit_sc(nxt)
            ctx = ctx_of[name, qh]
            for qi in range(2):
                nc.tensor.matmul(
                    ctx[:, qi * 512:(qi + 1) * 512],
                    lhsT=v_sb[:, kc, h * 65:h * 65 + 65],
                    rhs=ex[:, qi * 512:(qi + 1) * 512],
                    start=(kc == 0), stop=(kc == KC - 1))

        # deferred epilogue pieces: (due_step, fn)
        import collections as _c
        pending = _c.deque()

        def make_epilogue(st):
            name, qt, kt, pb, h, qh, kc = st
            cs = tmpB.tile([65, 1024], F32, tag="cs", bufs=2, name=f"cs{name}{qh}")

            def piece_copy():
                nc.vector.tensor_copy(cs[:, :], ctx_of[name, qh][:, :])

            def make_piece_half(half):
                def piece():
                    tp = psB.tile([128, 4, 65], F32, tag="s", bufs=2,
                                  name=f"tp{name}{qh}")
                    for j4 in range(4):
                        j = half * 4 + j4
                        nc.tensor.transpose(tp[:, j4, :],
                                            cs[:, j * 128:(j + 1) * 128],
                                            ident[0:65, 0:65])
                    for j4 in range(4):
                        jj = qh * 8 + half * 4 + j4
                        rd = tmpB.tile([128, 1], F32, tag="rd", bufs=4,
                                       name=f"rd{name}{qh}")
                        nc.vector.reciprocal(rd[:, :], tp[:, j4, 64:65])
                        nc.vector.tensor_scalar_mul(
                            out_sb[:, jj, h * 64:(h + 1) * 64],
                            tp[:, j4, 0:64], rd[:, :])
                return piece

            return [piece_copy, make_piece_half(0), make_piece_half(1)]

        emit_sc(steps[0])
        for i, st in enumerate(steps):
            nxt = steps[i + 1] if i + 1 < len(steps) else None
            emit_exp_pv(st, nxt)
            if st[6] == KC - 1:
                for k, piece in enumerate(make_epilogue(st)):
                    pending.append((i + 1 + k, piece))
            while pending and pending[0][0] <= i:
                pending.popleft()[1]()
        while pending:
            pending.popleft()[1]()

    for sc in range(KC):
        nc.sync.dma_start(d["out"][sc * 128:(sc + 1) * 128, :], out_sb[:, sc, :])

    outp.release()
    qkv.release()
    cst.release()


def _build():
    nc = bacc.Bacc("TRN2", target_bir_lowering=False, debug=False,
                   num_devices=NCORES)
    d = {}
    def dram(name, shape, out=False, dt=F32):
        d[name] = nc.dram_tensor(
            name, shape, dt,
            kind="ExternalOutput" if out else "ExternalInput").ap()
    dram("hsT", [HIDDEN, S], dt=F32R)
    dram("ctxT", [HIDDEN, S], dt=F32R)
    dram("wqT", [HIDDEN, RPC], dt=F32R)
    dram("wkT", [HIDDEN, RPC], dt=F32R)
    dram("wvT", [HIDDEN, RPC], dt=F32R)
    dram("bq", [RPC, 1])
    dram("bk", [RPC, 1])
    dram("bv", [RPC, 1])
    dram("mask", [128, KC])
    dram("ones", [128, KC * 3], dt=BF16)
    dram("out", [S, RPC], out=True)
    with tile.TileContext(nc) as tc:
        _emit(tc, nc, d)
    nc.compile()
    return nc


def _get_nc():
    global _cached_nc
    if _cached_nc is None:
        _cached_nc = _build()
    return _cached_nc


def make_in_maps(hidden_states, context, attention_mask, Wq, bq, Wk, bk, Wv, bv):
    f = lambda a: np.ascontiguousarray(np.asarray(a, dtype=np.float32))
    hs, cx, mask = f(hidden_states), f(context), f(attention_mask)
    Wq, Wk, Wv = f(Wq), f(Wk), f(Wv)
    bq, bk, bv = f(bq), f(bk), f(bv)
    in_maps = []
    for core in range(NCORES):
        b = core // 4
        g = core % 4
        rows = slice(g * RPC, (g + 1) * RPC)
        in_maps.append({
            "hsT": np.ascontiguousarray(hs[b].T),
            "ctxT": np.ascontiguousarray(cx[b].T),
            "wqT": np.ascontiguousarray(Wq[rows].T),
            "wkT": np.ascontiguousarray(Wk[rows].T),
            "wvT": np.ascontiguousarray(Wv[rows].T),
            "bq": np.ascontiguousarray(bq[rows].reshape(RPC, 1)),
            "bk": np.ascontiguousarray(bk[rows].reshape(RPC, 1)),
            "bv": np.ascontiguousarray(bv[rows].reshape(RPC, 1)),
            "mask": np.ascontiguousarray(mask[b, 0, 0, :].reshape(KC, 128).T),
            "ones": np.ones((128, KC * 3), dtype=_bf16np),
        })
    return in_maps


def gather_out(results):
    outs = [results[i]["out"] for i in range(NCORES)]
    return np.stack([np.concatenate([outs[b * 4 + g] for g in range(4)], axis=1)
                     for b in range(B)]).astype(np.float32)


def kernel(hidden_states, context, attention_mask, Wq, bq, Wk, bk, Wv, bv,
           trace=False):
    nc = _get_nc()
    in_maps = make_in_maps(hidden_states, context, attention_mask,
                           Wq, bq, Wk, bk, Wv, bv)
    res = run_bass_kernel_spmd(nc, in_maps, core_ids=list(range(NCORES)),
                               trace=trace)
    out = gather_out(res.results)
    if trace:
        kernel.last_results = res
    return out
